# revision 60
# baseline (speedup 1.0000x reference)
"""BiGRU encoder kernel for 8 Trainium2 NeuronCores.

Strategy (v4, fp8 DoubleRow + start-aligned streams):
  - Masked GRU over FIXED position ranges: forward runs positions (8-n)..7
    ascending, backward (6+n)..7 descending; a sample of length l starts at
    step n-l with h=0 (prefix memset); over-included samples (width padding)
    are frozen exactly at h=h_prev by forcing z=1 via a post-sigmoid
    max(z, mask01) on vector (no PSUM dependency).
  - Sort samples by window_len, deal round-robin to 8 cores; per core TWO
    batch tiles of 512 sorted samples -> 4 (tile, direction) streams, all
    START-aligned at superstep 0 (the DMA/latency-bound ramp has every
    stream in flight); ends spread by length so the short tile's MLP fires
    mid-schedule and the long tile's k-major W2 shortens the tail.
  - Step widths are EXACT per-step active counts (max over cores, rounded
    to 16 for alignment).
  - Matmuls: fp8e4 DoubleRow (K=256 per instruction) for ALL hidden
    projections and for input projections except the last N_HI_N=3 steps'
    n-gate, which runs in bf16 for accuracy.  Weights are pre-scaled by 512
    (exact power of 2) so unscaled fp8 x/h stay in e4m3's normal range;
    activations rescale with scale=1/512.
  - h is carried in bf16 (fp8 carry compounds error); the bf16 carry is
    written by vector (2x DVE mode), the fp8 copy for the next step's
    hidden matmuls by gpsimd (the next superstep absorbs its latency).
  - Hidden projections run at the PREVIOUS step's width; the n-gate
    pre-activation prefix (newly started samples) is r*bhh_n via a scalar
    ACT Copy with per-partition scale.
  - DMA order: step-0 x tiles and the first DR weight pair lead the
    sync/scalar queues (first matmul ~14us); bf16 weights follow at J0 on
    gpsimd; xb tiles ride the scalar queue.
  - Output is written feature-major [H, Bc] f32; host transposes and adds
    b2.
"""

import os
from contextlib import ExitStack

import numpy as np
import ml_dtypes

import concourse.bacc as bacc
import concourse.tile as tile
from concourse import mybir
from concourse.bass_utils import run_bass_kernel_spmd

NCORES = 8
B, T, D, H = 8192, 15, 512, 512
G = 3 * H
BIG = 40.0
TS = 512             # samples per batch tile
NTILES = 2
BC = TS * NTILES     # samples per core
N_HI_N = int(os.environ.get("GRU_NHI_N", "2"))   # n-gate bf16 on last-k steps
N_HI_RZ = int(os.environ.get("GRU_NHI_RZ", "0"))  # r/z bf16 on last-k steps
MLP_F8 = int(os.environ.get("GRU_MLP_F8", "0"))   # MLP matmuls in fp8 DR
F32 = mybir.dt.float32
BF16 = mybir.dt.bfloat16
F8 = mybir.dt.float8e4
DR = mybir.MatmulPerfMode.DoubleRow

ACT = mybir.ActivationFunctionType
ALU = mybir.AluOpType

NP_BF = ml_dtypes.bfloat16
NP_F8 = ml_dtypes.float8_e4m3

_PROGRAM_CACHE = {}
LAST_RESULT = None


def _build_program(sched):
    """sched[t][d] = tuple of (w, w_prev_hidden, mw, hi) per step."""
    nc = bacc.Bacc("TRN2", target_bir_lowering=False, debug=False,
                   num_devices=NCORES)

    w1dt = F8 if MLP_F8 >= 1 else BF16   # Linear1 dtype
    w2dt = F8 if MLP_F8 >= 2 else BF16   # Linear2 dtype
    x8_d = nc.dram_tensor("x8", [T, D, BC], F8, kind="ExternalInput")
    xb_d = nc.dram_tensor("xb", [T, D, BC], BF16, kind="ExternalInput")
    w8f_d = nc.dram_tensor("w8f", [D + H, G], F8, kind="ExternalInput")
    w8b_d = nc.dram_tensor("w8b", [D + H, G], F8, kind="ExternalInput")
    GB = G if N_HI_RZ else H  # bf16 weights: n-gate block only when rz=fp8
    wbf_d = nc.dram_tensor("wbf", [D, GB], BF16, kind="ExternalInput")
    wbb_d = nc.dram_tensor("wbb", [D, GB], BF16, kind="ExternalInput")
    w1_d = nc.dram_tensor("w1", [2 * H, H], w1dt, kind="ExternalInput")
    w2_d = nc.dram_tensor("w2", [H, H], w2dt, kind="ExternalInput")
    bias_d = nc.dram_tensor("bias", [40, 128], F32, kind="ExternalInput")
    mf_d = nc.dram_tensor("maskzf", [8, BC], BF16, kind="ExternalInput")
    mb_d = nc.dram_tensor("maskzb", [8, BC], BF16, kind="ExternalInput")
    y_d = nc.dram_tensor("y", [H, BC], F32, kind="ExternalOutput")

    NS = 2 * NTILES  # streams
    with tile.TileContext(nc) as tc, ExitStack() as ctx:
        const = ctx.enter_context(tc.tile_pool(name="const", bufs=1))
        x8pool = ctx.enter_context(tc.tile_pool(name="x8", bufs=10))
        xbpool = ctx.enter_context(tc.tile_pool(name="xb", bufs=8))
        hbf = [ctx.enter_context(tc.tile_pool(name=f"hb{s}", bufs=2))
               for s in range(NS)]
        hf8 = [ctx.enter_context(tc.tile_pool(name=f"h8{s}", bufs=2))
               for s in range(NS)]
        hfin = ctx.enter_context(tc.tile_pool(name="hfin", bufs=NS))
        gpool = ctx.enter_context(tc.tile_pool(name="g", bufs=26))
        hpool = ctx.enter_context(tc.tile_pool(name="mlph", bufs=2))
        mpool = ctx.enter_context(tc.tile_pool(name="m", bufs=8))
        opool = ctx.enter_context(tc.tile_pool(name="o", bufs=4))
        rzps = ctx.enter_context(tc.tile_pool(name="rz", bufs=4, space="PSUM"))
        xpps = ctx.enter_context(tc.tile_pool(name="xp", bufs=2, space="PSUM"))
        ghps = ctx.enter_context(tc.tile_pool(name="gh", bufs=2, space="PSUM"))

        def wtile(dram, kchunks, cols, dt, name):
            t_ = const.tile([128, kchunks, cols], dt, name=name)
            return t_, dram.rearrange("(c k) g -> k c g", k=128)

        w8f_t, w8f_s = wtile(w8f_d, 8, G, F8, "w8f")
        w8b_t, w8b_s = wtile(w8b_d, 8, G, F8, "w8b")
        wbf_t, wbf_s = wtile(wbf_d, 4, GB, BF16, "wbf")
        wbb_t, wbb_s = wtile(wbb_d, 4, GB, BF16, "wbb")
        NOFF = 2 * H if N_HI_RZ else 0  # n-gate column offset in wbf/wbb
        w1, w1_s = wtile(w1_d, 8, H, w1dt, "w1")
        w2, w2_s = wtile(w2_d, 4, H, w2dt, "w2")
        w8 = [w8f_t, w8b_t]
        wbf = [wbf_t, wbb_t]
        bt = const.tile([128, 40], F32)
        nc.gpsimd.dma_start(bt[:], bias_d.rearrange("n p -> p n"))
        mask_d = [mf_d, mb_d]

        # DMA order = order of first use.  Step-0 x tiles and masks are
        # emitted first on sync/gpsimd; the weights follow on sync+scalar
        # (parallel hw queues) so the first matmuls start as early as
        # possible.  bf16 input weights (mid-schedule hi steps) and MLP
        # weights trail on gpsimd/scalar.
        def load_weights_head():  # all input-proj chunks — ahead of x tiles
            # weights split over the scalar and gpsimd queues (~50 GB/s
            # each); the last chunk of each direction rides sync behind the
            # small step-0 x tiles so each queue moves ~590KB instead of 784
            for c in range(3):
                nc.scalar.dma_start(w8f_t[:, c, :], w8f_s[:, c, :])
                nc.gpsimd.dma_start(w8b_t[:, c, :], w8b_s[:, c, :])

        def load_weights_head2():  # after step-0 x tiles on sync
            nc.sync.dma_start(w8f_t[:, 3, :], w8f_s[:, 3, :])
            nc.sync.dma_start(w8b_t[:, 3, :], w8b_s[:, 3, :])

        def load_weights_first():  # bf16 input weights (short streams ~J1)
            for c in range(4):
                nc.scalar.dma_start(wbf_t[:, c, :], wbf_s[:, c, :])
                nc.gpsimd.dma_start(wbb_t[:, c, :], wbb_s[:, c, :])

        def load_late_weights_a():  # fp8 hidden chunks (needed from step 1)
            for c in range(4, 8):
                nc.scalar.dma_start(w8f_t[:, c, :], w8f_s[:, c, :])
                nc.gpsimd.dma_start(w8b_t[:, c, :], w8b_s[:, c, :])

        def load_late_weights_b():  # MLP weights (needed after short streams)
            for c in range(8):
                nc.scalar.dma_start(w1[:, c, :], w1_s[:, c, :])
            for c in range(4):
                nc.scalar.dma_start(w2[:, c, :], w2_s[:, c, :])

        state = {}

        def emit_step(t, d, j, steps):
            """Emit DMAs/tiles for one stream-step; return per-chunk emitter.
            Chunks of concurrent streams are interleaved by the caller so one
            stream's gate chain hides under other streams' matmuls."""
            w, wh, mw, hi_n, hi_rz = steps[j]
            n = len(steps)
            first = j == 0
            last = j == n - 1
            so = TS - w
            soh = TS - wh if not first else None  # hidden-proj suffix offset
            a0 = t * TS + so
            bb = 16 * d
            pos = (8 - n + j) if d == 0 else (6 + n - j)
            skey = 2 * t + d

            xtb = xt8 = None
            if hi_n:  # n-gate (and maybe r/z) input proj in bf16
                xtb = xbpool.tile([128, 4, TS], BF16, tag="xb", name="xt")
                nc.scalar.dma_start(
                    xtb[:, :, so:],
                    xb_d[pos].rearrange("(c k) s -> k c s", k=128)[:, :, a0:a0 + w])
            if not hi_rz:  # r/z input proj in fp8
                xt8 = x8pool.tile([128, 4, TS], F8, tag="x8", name="xt")
                nc.sync.dma_start(
                    xt8[:, :, so:],
                    x8_d[pos].rearrange("(c k) s -> k c s", k=128)[:, :, a0:a0 + w])
            mt = None
            if mw:
                mt = mpool.tile([128, 64], BF16, tag="m", name="mt")
                nc.gpsimd.dma_start(
                    mt[:, :mw],
                    mask_d[d][8 - (n - j), a0:a0 + mw].partition_broadcast(128))

            if first:
                h_prev = h8_prev = None
            else:
                h_prev, h8_prev, _ = state[skey]
            hdt = (F8 if MLP_F8 else BF16) if last else BF16
            hb_next = (hfin if last else hbf[skey]).tile(
                [128, 4, TS], hdt, tag="hf" if last else f"h{skey}", name="hb")
            h8_next = None
            if not last:
                h8_next = hf8[skey].tile([128, 4, TS], F8, tag=f"g8{skey}",
                                         name="h8")
                nw = steps[j + 1][0]
                if TS - nw < so:  # zero newly exposed prefix for next step's dd
                    nc.gpsimd.memset(hb_next[:, :, TS - nw:so].bitcast(F32), 0.0)
            state[skey] = (hb_next, h8_next, w)

            # narrow steps pack r|z (and xpn|ghn) into single PSUM banks so
            # twice as many chunk-calls can be in flight
            narrow = (w <= 256 and (first or wh <= 256)
                      and os.environ.get("GRU_NARROW"))

            def chunk(i):
                c0 = i * 128
                if narrow:
                    rzt = rzps.tile([128, 2, 256], F32, tag="rz",
                                    name=f"rzp{i}")
                    r_ps = rzt[:, 0, :w]
                    z_ps = rzt[:, 1, :w]
                    xgp = (xpps if i % 2 == 0 else ghps).tile(
                        [128, 2, 256], F32, tag="xp" if i % 2 == 0 else "gh",
                        name=f"xgp{i}")
                    xpn = xgp[:, 0, :w]
                else:
                    r_ps = rzps.tile([128, w], F32, tag="rz", name=f"rps{i}")
                    z_ps = rzps.tile([128, w], F32, tag="rz", name=f"zps{i}")
                    xpn = xpps.tile([128, w], F32, tag="xp", name=f"xpn{i}")
                # r/z input projections
                if hi_rz:
                    ww = wbf[d]
                    for k in range(4):
                        st = k == 0
                        lastk = k == 3 and first
                        nc.tensor.matmul(r_ps[:], ww[:, k, c0:c0 + 128],
                                         xtb[:, k, so:], start=st, stop=lastk)
                        nc.tensor.matmul(z_ps[:], ww[:, k, H + c0:H + c0 + 128],
                                         xtb[:, k, so:], start=st, stop=lastk)
                else:
                    ww = w8[d]
                    for p in range(2):
                        st = p == 0
                        lastk = p == 1 and first
                        ksl = slice(2 * p, 2 * p + 2)
                        nc.tensor.matmul(r_ps[:], ww[:, ksl, c0:c0 + 128],
                                         xt8[:, ksl, so:], start=st, stop=lastk,
                                         perf_mode=DR)
                        nc.tensor.matmul(z_ps[:], ww[:, ksl, H + c0:H + c0 + 128],
                                         xt8[:, ksl, so:], start=st, stop=lastk,
                                         perf_mode=DR)
                # n-gate input projection
                if hi_n:
                    ww = wbf[d]
                    for k in range(4):
                        nc.tensor.matmul(xpn[:], ww[:, k, NOFF + c0:NOFF + c0 + 128],
                                         xtb[:, k, so:], start=k == 0, stop=k == 3)
                else:
                    ww = w8[d]
                    for p in range(2):
                        ksl = slice(2 * p, 2 * p + 2)
                        nc.tensor.matmul(xpn[:], ww[:, ksl, 2 * H + c0:2 * H + c0 + 128],
                                         xt8[:, ksl, so:], start=p == 0, stop=p == 1,
                                         perf_mode=DR)
                ghn = None
                if not first:
                    wwh = w8[d]
                    if narrow:
                        ghn = xgp[:, 1, :wh]
                    else:
                        ghn = ghps.tile([128, wh], F32, tag="gh",
                                        name=f"ghn{i}")
                    for p in range(2):
                        ksl = slice(4 + 2 * p, 4 + 2 * p + 2)
                        nc.tensor.matmul(r_ps[:, soh - so:],
                                         wwh[:, ksl, c0:c0 + 128],
                                         h8_prev[:, 2 * p:2 * p + 2, soh:],
                                         start=False, stop=p == 1, perf_mode=DR)
                        nc.tensor.matmul(z_ps[:, soh - so:],
                                         wwh[:, ksl, H + c0:H + c0 + 128],
                                         h8_prev[:, 2 * p:2 * p + 2, soh:],
                                         start=False, stop=p == 1, perf_mode=DR)
                        nc.tensor.matmul(ghn[:],
                                         wwh[:, ksl, 2 * H + c0:2 * H + c0 + 128],
                                         h8_prev[:, 2 * p:2 * p + 2, soh:],
                                         start=p == 0, stop=p == 1, perf_mode=DR)

                r = gpool.tile([128, w], BF16, tag="g", name="r")
                nc.scalar.activation(r[:], r_ps[:], ACT.Sigmoid,
                                     bias=bt[:, bb + i:bb + i + 1],
                                     scale=1.0 / 512)
                z = gpool.tile([128, w], BF16, tag="g", name="z")
                nc.scalar.activation(z[:], z_ps[:], ACT.Sigmoid,
                                     bias=bt[:, bb + 4 + i:bb + 5 + i],
                                     scale=1.0 / 512)
                if mw:
                    # freeze not-yet-started samples: z=1 keeps h at h_prev
                    # exactly (h=0 until the true start step)
                    nc.vector.tensor_max(z[:, :mw], z[:, :mw], mt[:, :mw])

                # PSUM-freeing ops (STT reads ghn, add reads xpn) stay in
                # phase A so the xp/gh pools recycle at full rate
                tt = gpool.tile([128, w], BF16, tag="g", name="tt")
                if first:
                    # tt = r * bhh_n via ACT Copy with per-partition scale
                    nc.scalar.activation(tt[:], r[:], ACT.Copy, bias=0.0,
                                         scale=bt[:, bb + 8 + i:bb + 9 + i])
                else:
                    dd = soh - so
                    if dd:
                        nc.scalar.activation(tt[:, :dd], r[:, :dd],
                                             ACT.Copy, bias=0.0,
                                             scale=bt[:, bb + 8 + i:bb + 9 + i])
                    nc.vector.scalar_tensor_tensor(
                        tt[:, dd:], ghn[:], bt[:, bb + 8 + i:bb + 9 + i],
                        r[:, dd:], op0=ALU.add, op1=ALU.mult)
                ss = gpool.tile([128, w], BF16, tag="g", name="ss")
                nc.vector.tensor_add(ss[:], tt[:], xpn[:])

                def phase_b():
                    # tanh + h update (SBUF-only), optionally emitted one
                    # chunk-call later so the tanh never head-of-line-blocks
                    # the next call's r/z sigmoids on the scalar FIFO
                    nn = gpool.tile([128, w], BF16, tag="g", name="n")
                    nc.scalar.activation(nn[:], ss[:], ACT.Tanh,
                                         bias=bt[:, bb + 12 + i:bb + 13 + i],
                                         scale=1.0 / 512)
                    # bf16 carry on vector (2x mode); the fp8 copy for the
                    # next step's hidden matmuls goes to gpsimd (the next
                    # superstep is far enough out to absorb its latency)
                    ho = hb_next[:, i, so:]
                    if first:
                        e = gpool.tile([128, w], BF16, tag="g", name="e")
                        nc.vector.tensor_mul(e[:], z[:], nn[:])
                        nc.vector.tensor_sub(ho, nn[:], e[:])
                        if h8_next is not None:
                            nc.gpsimd.tensor_sub(h8_next[:, i, so:],
                                                 nn[:], e[:])
                    else:
                        dd_t = gpool.tile([128, w], BF16, tag="g", name="dd")
                        nc.vector.tensor_sub(dd_t[:], h_prev[:, i, so:], nn[:])
                        e = gpool.tile([128, w], BF16, tag="g", name="e")
                        nc.vector.tensor_mul(e[:], z[:], dd_t[:])
                        nc.vector.tensor_add(ho, nn[:], e[:])
                        if h8_next is not None:
                            nc.gpsimd.tensor_add(h8_next[:, i, so:],
                                                 nn[:], e[:])

                return phase_b

            return chunk, hb_next

        def emit_mlp(t, hf_t, hb_t):
            hidt = hpool.tile([128, 4, TS], F8 if MLP_F8 >= 2 else BF16,
                              tag="mh", name="hid")
            sc1 = 1.0 / 512 if MLP_F8 >= 1 else 1.0
            sc2 = 1.0 / 512 if MLP_F8 >= 2 else 1.0
            for i in range(4):
                ps = xpps.tile([128, TS], F32, tag="xp", name="mps")
                if MLP_F8 >= 1:
                    for p in range(4):
                        src = hf_t if p < 2 else hb_t
                        kc = (2 * p) % 4
                        nc.tensor.matmul(ps[:], w1[:, 2 * p:2 * p + 2,
                                                    i * 128:(i + 1) * 128],
                                         src[:, kc:kc + 2, :],
                                         start=p == 0, stop=p == 3, perf_mode=DR)
                else:
                    for k in range(8):
                        src = hf_t if k < 4 else hb_t
                        nc.tensor.matmul(ps[:], w1[:, k, i * 128:(i + 1) * 128],
                                         src[:, k % 4, :], start=k == 0,
                                         stop=k == 7)
                nc.scalar.activation(hidt[:, i, :], ps[:], ACT.Relu,
                                     bias=bt[:, 32 + i:33 + i], scale=sc1)

            def w2_mms(pst, i, kmajor_idx=None):
                if MLP_F8 >= 2:
                    for p in ([kmajor_idx] if kmajor_idx is not None
                              else range(2)):
                        nc.tensor.matmul(pst[:], w2[:, 2 * p:2 * p + 2,
                                                    i * 128:(i + 1) * 128],
                                         hidt[:, 2 * p:2 * p + 2, :],
                                         start=p == 0, stop=p == 1,
                                         perf_mode=DR)
                else:
                    for k in ([kmajor_idx] if kmajor_idx is not None
                              else range(4)):
                        nc.tensor.matmul(pst[:], w2[:, k, i * 128:(i + 1) * 128],
                                         hidt[:, k, :], start=k == 0,
                                         stop=k == (1 if MLP_F8 >= 2 else 3))

            # b2 is added host-side; output stays f32 (last tile DMAs the
            # PSUM directly, skipping the evacuation op entirely)
            # rotate output DMAs across all three queues — 256KB each takes
            # ~4us per queue, serial on one queue they'd trail the kernel
            oq = [nc.sync, nc.scalar, nc.gpsimd, nc.sync]
            if t == NTILES - 1:
                # k-major W2: accumulate into 4 PSUMs as relu chunks land,
                # shortening the end-of-kernel tail.
                pss = [rzps.tile([128, TS], F32, tag="rz", name=f"w2p{i}")
                       for i in range(4)]
                nk = 2 if MLP_F8 >= 2 else 4
                for k in range(nk):
                    for i in range(4):
                        w2_mms(pss[i], i, kmajor_idx=k)
                for i in range(4):
                    ob = opool.tile([128, TS], F32, tag="o", name="ob")
                    nc.vector.tensor_scalar(ob[:], pss[i][:], sc2, None,
                                            op0=ALU.mult)
                    oq[i].dma_start(
                        y_d[i * 128:(i + 1) * 128, t * TS:(t + 1) * TS],
                        ob[:])
            else:
                for i in range(4):
                    ps = xpps.tile([128, TS], F32, tag="xp", name="ops")
                    w2_mms(ps, i)
                    ob = opool.tile([128, TS], F32, tag="o", name="ob")
                    nc.vector.tensor_scalar(ob[:], ps[:], sc2, None,
                                            op0=ALU.mult)
                    oq[i].dma_start(
                        y_d[i * 128:(i + 1) * 128, t * TS:(t + 1) * TS],
                        ob[:])

        # End-staggered interleave: later (longer) quarters end later, so
        # every super-step has several streams in flight and the quarter
        # MLPs drain progressively instead of piling up at the end.
        # All streams start at J=0 so the latency-bound ramp phase has every
        # stream in flight; ends still spread by length (short tile's MLP
        # fires mid-schedule, long tile's at the end).
        starts = {}
        nmax = 0
        for t in range(NTILES):
            for d in range(2):
                starts[(t, d)] = 0
                nmax = max(nmax, len(sched[t][d]))

        hfs = {}
        mlp_done = set()
        load_weights_head()
        for J in range(nmax):
            chunk_fns = []
            for t in range(NTILES):
                for d in range(2):
                    steps = sched[t][d]
                    j = J - starts[(t, d)]
                    if 0 <= j < len(steps):
                        fn, h = emit_step(t, d, j, steps)
                        chunk_fns.append(fn)
                        if j == len(steps) - 1:
                            hfs[(t, d)] = h
            if J == 0:
                load_weights_head2()
                load_weights_first()
            lag = int(os.environ.get("GRU_LAG", "1"))
            smajor = not os.environ.get("GRU_CMAJOR")
            pending = []
            order = ([(fn, c) for fn in chunk_fns for c in range(4)]
                     if smajor else
                     [(fn, c) for c in range(4) for fn in chunk_fns])
            for fn, c in order:
                pb = fn(c)
                pending.append(pb)
                if len(pending) > lag:
                    pending.pop(0)()
            for pb in pending:
                pb()
            if J == 0:
                load_late_weights_a()
            elif J == 2:
                load_late_weights_b()
            for t in range(NTILES):
                if t not in mlp_done and (t, 0) in hfs and (t, 1) in hfs:
                    emit_mlp(t, hfs[(t, 0)], hfs[(t, 1)])
                    mlp_done.add(t)

    nc.compile()
    return nc


def _mk_sched(lens_pc, t):
    """lens_pc: [BC, NCORES] per-core sorted lengths; tile t rows."""
    seg = lens_pc[t * TS:(t + 1) * TS]  # [TS, NCORES]
    n = int(seg.max())
    steps = []
    for j in range(n):
        need = n - j
        cnt = (seg >= need).sum(axis=0)
        w = min(TS, -(-int(cnt.max()) // 16) * 16)
        mw = int(w - int(cnt.min()))
        hi_n = j >= n - N_HI_N
        hi_rz = j >= n - N_HI_RZ
        steps.append([w, 0, mw, hi_n, hi_rz])
    for j in range(1, n):
        steps[j][1] = steps[j - 1][0]  # hidden width = prev step width
    return tuple(tuple(s) for s in steps)


def kernel(padded_window, window_len, Wih_f, Whh_f, bih_f, bhh_f,
           Wih_b, Whh_b, bih_b, bhh_b, W1, b1, W2, b2):
    wl = np.asarray(window_len)
    lf = (wl - 1) // 2 + 1
    lb = wl // 2 + 1
    order = np.argsort(wl, kind="stable")

    lf_pc = lf[order].reshape(-1, NCORES)
    lb_pc = lb[order].reshape(-1, NCORES)

    sched = tuple((_mk_sched(lf_pc, t), _mk_sched(lb_pc, t))
                  for t in range(NTILES))

    if sched not in _PROGRAM_CACHE:
        _PROGRAM_CACHE[sched] = _build_program(sched)
    nc = _PROGRAM_CACHE[sched]

    f32 = np.float32
    wf_full = np.concatenate([Wih_f.T, Whh_f.T], 0).astype(f32) * 512.0
    wb_full = np.concatenate([Wih_b.T, Whh_b.T], 0).astype(f32) * 512.0
    w8f = np.clip(wf_full, -240, 240).astype(NP_F8)
    w8b = np.clip(wb_full, -240, 240).astype(NP_F8)
    if N_HI_RZ:
        wbf = wf_full[:D].astype(NP_BF)
        wbb = wb_full[:D].astype(NP_BF)
    else:  # only the n-gate block is ever read in bf16
        wbf = np.ascontiguousarray(wf_full[:D, 2 * H:]).astype(NP_BF)
        wbb = np.ascontiguousarray(wb_full[:D, 2 * H:]).astype(NP_BF)
    w1f = np.ascontiguousarray(W1.T, dtype=f32)
    w2f = np.ascontiguousarray(W2.T, dtype=f32)
    w1 = (np.clip(w1f * 512.0, -240, 240).astype(NP_F8) if MLP_F8 >= 1
          else w1f.astype(NP_BF))
    w2 = (np.clip(w2f * 512.0, -240, 240).astype(NP_F8) if MLP_F8 >= 2
          else w2f.astype(NP_BF))

    def chunks(v):
        return np.asarray(v, f32).reshape(4, 128)

    bias = np.concatenate([
        chunks((bih_f + bhh_f)[:H]), chunks((bih_f + bhh_f)[H:2 * H]),
        chunks(bhh_f[2 * H:] * 512.0), chunks(bih_f[2 * H:]),
        chunks((bih_b + bhh_b)[:H]), chunks((bih_b + bhh_b)[H:2 * H]),
        chunks(bhh_b[2 * H:] * 512.0), chunks(bih_b[2 * H:]),
        chunks(b1), chunks(b2),
    ], 0)  # [40, 128]

    pw = np.asarray(padded_window, f32)
    in_maps = []
    p8 = np.arange(8)
    for c in range(NCORES):
        idx = order[c::NCORES]
        xT = np.ascontiguousarray(pw[idx].transpose(1, 2, 0))  # [15, 512, BC]
        mzf = (p8[:, None] < (8 - lf[idx])[None, :]).astype(NP_BF)
        mzb = (p8[:, None] < (8 - lb[idx])[None, :]).astype(NP_BF)
        in_maps.append({
            "x8": np.clip(xT, -240, 240).astype(NP_F8),
            "xb": xT.astype(NP_BF),
            "w8f": w8f, "w8b": w8b, "wbf": wbf, "wbb": wbb,
            "w1": w1, "w2": w2, "bias": bias,
            "maskzf": mzf, "maskzb": mzb,
        })

    trace = bool(os.environ.get("GRU_TRACE"))
    kw = {}
    if os.environ.get("GRU_TMPDIR"):
        kw["tmpdir"] = os.environ["GRU_TMPDIR"]
    res = run_bass_kernel_spmd(nc, in_maps, core_ids=list(range(NCORES)),
                               trace=trace, **kw)
    global LAST_RESULT
    LAST_RESULT = res
    out = np.empty((B, H), f32)
    for c in range(NCORES):
        out[order[c::NCORES]] = np.asarray(res.results[c]["y"], f32).T
    out += np.asarray(b2, f32)[None, :]
    return out

